# revision 3
# baseline (speedup 1.0000x reference)
"""Trainium2 Bass kernel for nn_Aggregator (gnn_message_passing).

Math (reference):
  yes_skip  = skip_decisions with diagonal zeroed
  no_skip   = diag(skip_decisions)
  p_bt      = ip * no_skip * branch[:,0];  p_bf = ip * no_skip * branch[:,1]
  new_ip[j] = seg_sum(p_bt, true_idx)[j] + seg_sum(p_bf, false_idx)[j]
            + sum_i ip[i]*yes_skip[i,j]
  num[j,h]  = seg_sum(h*p_bt, true_idx)[j,h] + seg_sum(h*p_bf, false_idx)[j,h]
            + sum_i hssp[i,j,h]*ip[i]*yes_skip[i,j]
  new_hid   = num / (new_ip + 1e-7)

Distribution: i (source-node) axis sharded over 8 cores (128 rows each).
Each core computes partial sums over its i rows; two ReduceScatters
combine them so core c owns output rows [128c, 128c+128); host concats.

The dominant cost is streaming the [1024,1024,256] f32 (1 GiB) tensor;
per core 128 MiB at the ~358 GB/s HBM/NC limit => ~375 us roofline.
The weighted accumulation runs as fused scalar_tensor_tensor ops
(acc = tile*w_col + acc), split between the Vector and GpSimd engines.
"""
import sys

if '/opt/trn_rl_repo' not in sys.path:
    sys.path.insert(0, '/opt/trn_rl_repo')

import numpy as np

N = 1024
H = 256
NCORES = 8
S = N // NCORES        # i rows per core
P = 128                # partition tile size for the j axis

_prog_cache = {}


def build_program(n=N, h=H, ncores=NCORES, vec_jts=8, row_bufs=6):
    """Build the SPMD Bass program (same program for every core).

    vec_jts: j-tiles handled by the Vector engine; the rest go to GpSimd.
    """
    import concourse.bass as bass  # noqa: F401  (registers engine classes)
    import concourse.bacc as bacc
    import concourse.mybir as mybir
    from concourse import tile

    f32 = mybir.dt.float32
    s = n // ncores            # i rows per core
    jt_n = n // P              # j tiles of 128
    assert n % P == 0 and s <= 128

    nc = bacc.Bacc(None)

    # ---- I/O ----
    hssp = nc.declare_dram_parameter("hssp", [s, n, h], f32, isOutput=False)
    skip = nc.declare_dram_parameter("skip", [s, n], f32, isOutput=False)
    ip = nc.declare_dram_parameter("ip", [s, 1], f32, isOutput=False)
    hprop = nc.declare_dram_parameter("hprop", [s, h], f32, isOutput=False)
    pt = nc.declare_dram_parameter("pt", [s, 1], f32, isOutput=False)
    pf = nc.declare_dram_parameter("pf", [s, 1], f32, isOutput=False)
    tidx = nc.declare_dram_parameter("tidx", [s, 1], f32, isOutput=False)
    fidx = nc.declare_dram_parameter("fidx", [s, 1], f32, isOutput=False)
    gidx = nc.declare_dram_parameter("gidx", [s, 1], f32, isOutput=False)
    iota = nc.declare_dram_parameter("iota", [s, n], f32, isOutput=False)
    ident = nc.declare_dram_parameter("ident", [s, s], f32, isOutput=False)
    out_hid = nc.declare_dram_parameter("out_hid", [s, h], f32, isOutput=True)
    out_ip = nc.declare_dram_parameter("out_ip", [s, 1], f32, isOutput=True)

    eq = mybir.AluOpType.is_equal
    mult = mybir.AluOpType.mult
    sub = mybir.AluOpType.subtract
    add = mybir.AluOpType.add

    with tile.TileContext(nc) as tc:
        with tc.tile_pool(name="dram", bufs=1, space="DRAM") as dram, \
             tc.tile_pool(name="const", bufs=1) as const, \
             tc.tile_pool(name="acc", bufs=1) as accp, \
             tc.tile_pool(name="rows", bufs=row_bufs) as rows, \
             tc.tile_pool(name="psum_t", bufs=2, space="PSUM") as psum_t, \
             tc.tile_pool(name="psum_h", bufs=2, space="PSUM") as psum_hp, \
             tc.tile_pool(name="psum_ip", bufs=2, space="PSUM") as psum_ipp:

            # collective buffers (internal DRAM)
            hidbuf = dram.tile([n, h], f32)
            ipbuf = dram.tile([n, 1], f32)
            hidscat = dram.tile([s, h], f32)
            ipscat = dram.tile([s, 1], f32)

            # ---- load small inputs ----
            skip_t = const.tile([s, n], f32, tag="skip")
            iota_t = const.tile([s, n], f32, tag="iota")
            ip_t = const.tile([s, 1], f32, tag="ip")
            h_t = const.tile([s, h], f32, tag="h")
            pt_t = const.tile([s, 1], f32, tag="pt")
            pf_t = const.tile([s, 1], f32, tag="pf")
            tidx_t = const.tile([s, 1], f32, tag="tidx")
            fidx_t = const.tile([s, 1], f32, tag="fidx")
            gidx_t = const.tile([s, 1], f32, tag="gidx")
            ident_t = const.tile([s, s], f32, tag="ident")
            nc.sync.dma_start(skip_t[:], skip[:])
            nc.sync.dma_start(iota_t[:], iota[:])
            nc.sync.dma_start(ip_t[:], ip[:])
            nc.sync.dma_start(h_t[:], hprop[:])
            nc.sync.dma_start(pt_t[:], pt[:])
            nc.sync.dma_start(pf_t[:], pf[:])
            nc.sync.dma_start(tidx_t[:], tidx[:])
            nc.sync.dma_start(fidx_t[:], fidx[:])
            nc.sync.dma_start(gidx_t[:], gidx[:])
            nc.sync.dma_start(ident_t[:], ident[:])

            # ---- stage A: small tensors ----
            # diagonal one-hot D, no_skip, w = ip * yes_skip
            d_t = const.tile([s, n], f32, tag="d")
            scr_t = const.tile([s, n], f32, tag="scr")
            w_t = const.tile([s, n], f32, tag="w")
            nsk_t = const.tile([s, 1], f32, tag="nsk")
            nc.vector.tensor_scalar(d_t[:], iota_t[:], gidx_t[:], None, op0=eq)
            nc.vector.tensor_tensor(scr_t[:], skip_t[:], d_t[:], mult)
            nc.vector.reduce_sum(nsk_t[:], scr_t[:], mybir.AxisListType.X)
            nc.vector.tensor_tensor(w_t[:], skip_t[:], scr_t[:], sub)
            nc.vector.tensor_scalar_mul(w_t[:], w_t[:], ip_t[:])

            # p_branch_true/false (per-partition scalars)
            pbt_t = const.tile([s, 1], f32, tag="pbt")
            pbf_t = const.tile([s, 1], f32, tag="pbf")
            tmp1_t = const.tile([s, 1], f32, tag="tmp1")
            nc.vector.tensor_tensor(tmp1_t[:], ip_t[:], nsk_t[:], mult)
            nc.vector.tensor_tensor(pbt_t[:], tmp1_t[:], pt_t[:], mult)
            nc.vector.tensor_tensor(pbf_t[:], tmp1_t[:], pf_t[:], mult)

            # one-hot segment matrices
            tmat_t = const.tile([s, n], f32, tag="tmat")
            fmat_t = const.tile([s, n], f32, tag="fmat")
            nc.vector.tensor_scalar(tmat_t[:], iota_t[:], tidx_t[:], None, op0=eq)
            nc.vector.tensor_scalar(fmat_t[:], iota_t[:], fidx_t[:], None, op0=eq)

            # weighted hidden proposals
            pht_t = const.tile([s, h], f32, tag="pht")
            phf_t = const.tile([s, h], f32, tag="phf")
            nc.vector.tensor_scalar_mul(pht_t[:], h_t[:], pbt_t[:])
            nc.vector.tensor_scalar_mul(phf_t[:], h_t[:], pbf_t[:])

            # ones for the skip_contrib column-sum matmul
            ones_t = const.tile([s, 1], f32, tag="ones")
            nc.vector.memset(ones_t[:], 1.0)

            # wT tiles: [128 j, s i] per j-tile, via PE transpose
            wT = []
            for jt in range(jt_n):
                pt_ps = psum_t.tile([P, s], f32)
                nc.tensor.transpose(pt_ps[:], w_t[:, jt * P:(jt + 1) * P], ident_t[:])
                wt_t = accp.tile([P, s], f32, tag=f"wT{jt}")
                nc.scalar.copy(wt_t[:], pt_ps[:])
                wT.append(wt_t)

            # ip partials: new_ip_part[j] = T'p_bt + F'p_bf + w'1
            ip_part = const.tile([1, n], f32, tag="ip_part")
            ck = min(512, n)
            for c0 in range(0, n, ck):
                ps = psum_ipp.tile([1, ck], f32)
                nc.tensor.matmul(ps[:], pbt_t[:], tmat_t[:, c0:c0 + ck],
                                 start=True, stop=False)
                nc.tensor.matmul(ps[:], pbf_t[:], fmat_t[:, c0:c0 + ck],
                                 start=False, stop=False)
                nc.tensor.matmul(ps[:], ones_t[:], w_t[:, c0:c0 + ck],
                                 start=False, stop=True)
                nc.scalar.copy(ip_part[:, c0:c0 + ck], ps[:])
            nc.sync.dma_start(ipbuf[:], ip_part[:])

            # hidden segment-sum partials; initialize stage-B accumulators
            acc = []
            for jt in range(jt_n):
                ps = psum_hp.tile([P, h], f32)
                nc.tensor.matmul(ps[:], tmat_t[:, jt * P:(jt + 1) * P], pht_t[:],
                                 start=True, stop=False)
                nc.tensor.matmul(ps[:], fmat_t[:, jt * P:(jt + 1) * P], phf_t[:],
                                 start=False, stop=True)
                a_t = accp.tile([P, h], f32, tag=f"acc{jt}")
                nc.scalar.copy(a_t[:], ps[:])
                acc.append(a_t)

            # ---- stage B: stream hssp, fused multiply-accumulate ----
            hssp_r = hssp.rearrange("i (t p) h -> i p t h", p=P)
            for i in range(s):
                row = rows.tile([P, jt_n * h], f32, tag="row")
                row_v = row[:].rearrange("p (t h) -> p t h", h=h)
                nc.sync.dma_start(row_v, hssp_r[i])
                for jt in range(jt_n):
                    eng = nc.vector if jt < vec_jts else nc.gpsimd
                    eng.scalar_tensor_tensor(
                        acc[jt][:],
                        row[:, jt * h:(jt + 1) * h],
                        wT[jt][:, i:i + 1],
                        acc[jt][:],
                        op0=mult,
                        op1=add,
                    )

            # ---- stage C: combine across cores, divide, output ----
            for jt in range(jt_n):
                nc.sync.dma_start(hidbuf[jt * P:(jt + 1) * P, :], acc[jt][:])
            nc.gpsimd.collective_compute(
                "ReduceScatter", add,
                ins=[hidbuf.opt()], outs=[hidscat.opt()],
                replica_groups=[list(range(ncores))],
            )
            nc.gpsimd.collective_compute(
                "ReduceScatter", add,
                ins=[ipbuf.opt()], outs=[ipscat.opt()],
                replica_groups=[list(range(ncores))],
            )

            hs_t = const.tile([s, h], f32, tag="hs")
            ips_t = const.tile([s, 1], f32, tag="ips")
            den_t = const.tile([s, 1], f32, tag="den")
            rec_t = const.tile([s, 1], f32, tag="rec")
            nc.sync.dma_start(hs_t[:], hidscat[:])
            nc.sync.dma_start(ips_t[:], ipscat[:])
            nc.vector.tensor_scalar_add(den_t[:], ips_t[:], 1e-7)
            nc.vector.reciprocal(rec_t[:], den_t[:])
            nc.vector.tensor_scalar_mul(hs_t[:], hs_t[:], rec_t[:])
            nc.sync.dma_start(out_hid[:], hs_t[:])
            nc.sync.dma_start(out_ip[:], ips_t[:])

    nc.finalize()
    return nc


def make_in_maps(inputs, n=N, ncores=NCORES):
    """Shard the full inputs into per-core input maps."""
    s = n // ncores
    ipf = np.asarray(inputs["instruction_pointer"], dtype=np.float32)
    hp = np.asarray(inputs["hidden_state_proposals"], dtype=np.float32)
    hssp = np.asarray(inputs["hidden_state_skip_proposals"], dtype=np.float32)
    sk = np.asarray(inputs["skip_decisions"], dtype=np.float32)
    br = np.asarray(inputs["branch_decisions"], dtype=np.float32)
    ti = np.asarray(inputs["true_indexes"])
    fi = np.asarray(inputs["false_indexes"])

    iota = np.broadcast_to(np.arange(n, dtype=np.float32), (s, n))
    iota = np.ascontiguousarray(iota)
    ident = np.eye(s, dtype=np.float32)

    maps = []
    for c in range(ncores):
        lo, hi = c * s, (c + 1) * s
        maps.append({
            "hssp": hssp[lo:hi],
            "skip": sk[lo:hi],
            "ip": ipf[lo:hi].reshape(s, 1),
            "hprop": hp[lo:hi],
            "pt": np.ascontiguousarray(br[lo:hi, 0:1]),
            "pf": np.ascontiguousarray(br[lo:hi, 1:2]),
            "tidx": ti[lo:hi].astype(np.float32).reshape(s, 1),
            "fidx": fi[lo:hi].astype(np.float32).reshape(s, 1),
            "gidx": np.arange(lo, hi, dtype=np.float32).reshape(s, 1),
            "iota": iota,
            "ident": ident,
        })
    return maps


def kernel(**inputs):
    from concourse.bass_utils import run_bass_kernel_spmd

    key = "full"
    if key not in _prog_cache:
        _prog_cache[key] = build_program()
    nc = _prog_cache[key]

    in_maps = make_in_maps(inputs)
    res = run_bass_kernel_spmd(nc, in_maps, list(range(NCORES)))
    new_ip = np.concatenate(
        [res.results[c]["out_ip"].reshape(-1) for c in range(NCORES)])
    new_hid = np.concatenate(
        [res.results[c]["out_hid"] for c in range(NCORES)], axis=0)
    return new_ip, new_hid
